# revision 11
# baseline (speedup 1.0000x reference)
"""Memory-optimized MLA (multi-head latent attention) Bass kernel for 8 trn2 cores.

Sharding: DP=2 over batch x TP=4 over heads (2 heads/core).
core c -> batch b=c//4, head-group g=c%4 (global heads 2g, 2g+1).

Per-core device pipeline (all layouts transposed: [feature, token]):
  A: down-proj  c_kvT/c_qT/k_rotT = W_down^T @ h^T      (K=d_model, 40 chunks)
  B: up-proj    qT/kT [640,2048], v [2048,640] per head  (K=d_kv=128)  + RoPE
  C: attention  S^T = kT^T qT -> exp -> PV + ones-row normalizer (no-max softmax)
  AG: AllGather normalized attn_out^T slices over the 4 TP ranks (bf16)
  E: out[tok, e_slice] = attn_full^T^T @ W_o[:, e_slice] + b_o   (bf16 matmul)
Host gathers the 8 [2048,1280] slices into [2,2048,5120].
"""
import sys
if '/opt/trn_rl_repo' not in sys.path:
    sys.path.insert(0, '/opt/trn_rl_repo')
import numpy as np
import ml_dtypes

D_MODEL = 5120
N_HEADS = 8
D_HEAD = 640
D_ROPE = 16
SPLIT = 624
D_KV = 128
BATCH, SEQ = 2, 2048
ROPE_SCALE = 40.0
SCALE = 1.0 / np.sqrt(np.float32(D_HEAD))

N_CORES = 8
TP = 4           # head-parallel ranks per batch group
HL = 2           # local heads per core
ESL = D_MODEL // TP          # 1280 output cols per core
DM_CH = D_MODEL // 128       # 40
TOKT = SEQ // 128            # 16 token tiles
TOKC = SEQ // 512            # 4 token chunks

_CACHE = {}
LAST_RESULTS = None


def _build_nc():
    import concourse.bacc as bacc
    import concourse.mybir as mybir
    import concourse.tile as tile

    f32 = mybir.dt.float32
    f32r = mybir.dt.float32r
    bf16 = mybir.dt.bfloat16
    Exp = mybir.ActivationFunctionType.Exp
    Copy = mybir.ActivationFunctionType.Copy
    Ident = mybir.ActivationFunctionType.Identity

    nc = bacc.Bacc("TRN2", target_bir_lowering=False, debug=False,
                   num_devices=N_CORES)

    hT = nc.dram_tensor("hT", [D_MODEL, SEQ], f32r, kind="ExternalInput").ap()
    Wdown = nc.dram_tensor("Wdown", [D_MODEL, 384], f32r, kind="ExternalInput").ap()
    down_bias = nc.dram_tensor("down_bias", [128, 3], f32, kind="ExternalInput").ap()
    Wq = nc.dram_tensor("Wq", [D_KV, HL * 640], f32r, kind="ExternalInput").ap()
    Wk = nc.dram_tensor("Wk", [D_KV, HL * 624], f32r, kind="ExternalInput").ap()
    Wv = nc.dram_tensor("Wv", [D_KV, HL * 640], f32r, kind="ExternalInput").ap()
    qk_bias = nc.dram_tensor("qk_bias", [128, 20], f32, kind="ExternalInput").ap()
    vbias = nc.dram_tensor("vbias", [128, HL * 640], f32, kind="ExternalInput").ap()
    costab = nc.dram_tensor("costab", [16, SEQ], f32r, kind="ExternalInput").ap()
    sintab = nc.dram_tensor("sintab", [16, SEQ], f32r, kind="ExternalInput").ap()
    ones_in = nc.dram_tensor("ones", [128, 1], f32r, kind="ExternalInput").ap()
    Wo = nc.dram_tensor("Wo", [D_MODEL, ESL], bf16, kind="ExternalInput").ap()
    bo = nc.dram_tensor("bo", [128, ESL], f32, kind="ExternalInput").ap()
    out = nc.dram_tensor("out", [SEQ, ESL], f32, kind="ExternalOutput").ap()

    from contextlib import ExitStack
    with tile.TileContext(nc) as tc:
        # ---------- constants ----------
        with ExitStack() as _stk:
            dram = _stk.enter_context(
                tc.tile_pool(name="dram", bufs=1, space="DRAM"))
            _inner = ExitStack()
            cst = _inner.enter_context(tc.tile_pool(name="const", bufs=1))
            cTp = _inner.enter_context(tc.tile_pool(name="cT", bufs=1))
            dbias_t = cst.tile([128, 3], f32, name="dbias_t")
            nc.sync.dma_start(dbias_t[:], down_bias)
            qkb_t = cst.tile([128, 20], f32, name="qkb_t")
            nc.sync.dma_start(qkb_t[:], qk_bias)
            vb_t = cst.tile([128, HL * 640], f32, name="vb_t")
            nc.sync.dma_start(vb_t[:], vbias)
            ones_t = cst.tile([128, 1], f32r, name="ones_t")
            nc.sync.dma_start(ones_t[:], ones_in)
            # rope tables parked at partitions 112:128 (both rope sites land there)
            cs_t = cst.tile([128, SEQ], f32r, name="cs_t")
            sn_t = cst.tile([128, SEQ], f32r, name="sn_t")
            nc.sync.dma_start(cs_t[0:16, :], costab)
            nc.sync.dma_start(sn_t[0:16, :], sintab)

            c_kvT = cTp.tile([128, SEQ], f32r, name="c_kvT")
            c_qT = cTp.tile([128, SEQ], f32r, name="c_qT")
            k_rotT = cTp.tile([128, SEQ], f32r, name="k_rotT")
            cT = [c_kvT, c_qT, k_rotT]

            ag_in = dram.tile([TP * 1280 // TP, SEQ], bf16, name="ag_in")
            ag_out = dram.tile([D_MODEL, SEQ], bf16, name="ag_out")

            # ---------- phase A: down projections ----------
            with tc.tile_pool(name="wdown", bufs=1) as wdp, \
                 tc.tile_pool(name="hstream", bufs=4) as hsp, \
                 tc.tile_pool(name="psA", bufs=6, space="PSUM") as psA:
                wd = []
                for i in range(DM_CH):
                    w = wdp.tile([128, 384], f32r, name=f"wd{i}")
                    nc.sync.dma_start(w[:], Wdown[i * 128:(i + 1) * 128, :])
                    wd.append(w)
                for tc4 in range(TOKC):
                    ps = [psA.tile([128, 512], f32, tag="psA", name=f"psA_{tc4}_{m}")
                          for m in range(3)]
                    for dm in range(DM_CH):
                        ht = hsp.tile([128, 512], f32r, tag="ht", name=f"ht{tc4}_{dm}")
                        nc.sync.dma_start(
                            ht[:], hT[dm * 128:(dm + 1) * 128,
                                      tc4 * 512:(tc4 + 1) * 512])
                        for m in range(3):
                            nc.tensor.matmul(
                                ps[m][:], wd[dm][:, m * 128:(m + 1) * 128],
                                ht[:], start=(dm == 0), stop=(dm == DM_CH - 1),
                                skip_group_check=True)
                    for m in range(3):
                        nc.scalar.activation(
                            cT[m][:, tc4 * 512:(tc4 + 1) * 512], ps[m][:],
                            Ident, bias=dbias_t[:, m:m + 1])

            # ---------- phases B+C per head ----------
            with tc.tile_pool(name="upw", bufs=1) as upw, \
                 tc.tile_pool(name="qkv", bufs=1) as qkv, \
                 tc.tile_pool(name="expp", bufs=3) as expp, \
                 tc.tile_pool(name="ev", bufs=3) as evp, \
                 tc.tile_pool(name="ps_s", bufs=2, space="PSUM") as ps_s, \
                 tc.tile_pool(name="ps_pv", bufs=1, space="PSUM") as ps_pv:
                for hl in range(HL):
                    wq_t = upw.tile([128, 640], f32r, tag="wq", name=f"wq{hl}")
                    nc.sync.dma_start(wq_t[:], Wq[:, hl * 640:(hl + 1) * 640])
                    wk_t = upw.tile([128, 624], f32r, tag="wk", name=f"wk{hl}")
                    nc.sync.dma_start(wk_t[:], Wk[:, hl * 624:(hl + 1) * 624])
                    wv_t = upw.tile([128, 640], f32r, tag="wv", name=f"wv{hl}")
                    nc.sync.dma_start(wv_t[:], Wv[:, hl * 640:(hl + 1) * 640])

                    qT = [qkv.tile([128, SEQ], f32r, tag=f"qT{m}", name=f"qT{m}_{hl}")
                          for m in range(5)]
                    kT = [qkv.tile([128, SEQ], f32r, tag=f"kT{m}", name=f"kT{m}_{hl}")
                          for m in range(5)]
                    v = [qkv.tile([128, D_HEAD], f32r, tag=f"v{t}", name=f"v{t}_{hl}")
                         for t in range(TOKT)]

                    # q up-proj (5 chunks of 128 feats; last 16 rows are q_rot)
                    for m in range(5):
                        for tc4 in range(TOKC):
                            ps = ps_s.tile([128, 512], f32, tag="s",
                                           name=f"psq{hl}_{m}_{tc4}")
                            nc.tensor.matmul(
                                ps[:], wq_t[:, m * 128:(m + 1) * 128],
                                c_qT[:, tc4 * 512:(tc4 + 1) * 512],
                                start=True, stop=True, skip_group_check=True)
                            nc.scalar.activation(
                                qT[m][:, tc4 * 512:(tc4 + 1) * 512], ps[:],
                                Ident, bias=qkb_t[:, hl * 5 + m:hl * 5 + m + 1],
                                scale=float(SCALE))
                    # k up-proj (624 feats: 4x128 + 112)
                    for m in range(5):
                        mw = 128 if m < 4 else 112
                        for tc4 in range(TOKC):
                            ps = ps_s.tile([128, 512], f32, tag="s",
                                           name=f"psk{hl}_{m}_{tc4}")
                            nc.tensor.matmul(
                                ps[:mw, :], wk_t[:, m * 128:m * 128 + mw],
                                c_kvT[:, tc4 * 512:(tc4 + 1) * 512],
                                start=True, stop=True, skip_group_check=True)
                            nc.scalar.activation(
                                kT[m][:mw, tc4 * 512:(tc4 + 1) * 512], ps[:mw, :],
                                Ident, bias=qkb_t[:mw, 10 + hl * 5 + m:11 + hl * 5 + m])
                    # k_rot rows -> kT[4][112:128]
                    h_glob = None  # head index resolved on host; rows via input layout
                    # k_rotT rows for this local head sit at [ (2g+hl)*16 : +16 ] --
                    # host packs k_rotT for ALL heads; we need global head = 2g+hl,
                    # but g differs per core. Host instead rotates W_kr columns so
                    # that OUR two heads occupy rows 0:16 (hl=0) and 16:32 (hl=1).
                    nc.sync.dma_start(kT[4][112:128, :],
                                      k_rotT[hl * 16:(hl + 1) * 16, :])
                    # v up-proj: [tok, d] orientation
                    for t in range(TOKT):
                        for half in range(2):
                            ps = ps_s.tile([128, 320], f32, tag="s",
                                           name=f"psv{hl}_{t}_{half}")
                            nc.tensor.matmul(
                                ps[:], c_kvT[:, t * 128:(t + 1) * 128],
                                wv_t[:, half * 320:(half + 1) * 320],
                                start=True, stop=True, skip_group_check=True)
                            nc.vector.tensor_add(
                                v[t][:, half * 320:(half + 1) * 320], ps[:],
                                vb_t[:, hl * 640 + half * 320:hl * 640 + (half + 1) * 320])

                    # RoPE on qT[4][112:128] / kT[4][112:128]: DVE needs
                    # 32-aligned partition bases, so stage rows at 0:16.
                    for name, x in (("q", qT[4]), ("k", kT[4])):
                        for cc in range(TOKC):
                            sl = slice(cc * 512, (cc + 1) * 512)
                            stg = expp.tile([128, 512], f32r, tag="expS",
                                            name=f"ropes_{name}{hl}_{cc}")
                            tmp = expp.tile([128, 512], f32r, tag="expS",
                                            name=f"ropet_{name}{hl}_{cc}")
                            nc.sync.dma_start(stg[0:16, 0:512], x[112:128, sl])
                            nc.sync.dma_start(tmp[0:4, 0:512], stg[4:8, 0:512])
                            nc.sync.dma_start(tmp[4:8, 0:512], stg[0:4, 0:512])
                            nc.vector.tensor_mul(tmp[0:8, 0:512], tmp[0:8, 0:512],
                                                 sn_t[0:8, sl])
                            nc.vector.tensor_mul(stg[0:8, 0:512], stg[0:8, 0:512],
                                                 cs_t[0:8, sl])
                            nc.vector.tensor_add(stg[0:8, 0:512], stg[0:8, 0:512],
                                                 tmp[0:8, 0:512])
                            nc.sync.dma_start(x[112:128, sl], stg[0:16, 0:512])

                    # ---------- phase C: attention ----------
                    for qc in range(TOKC):
                        qsl = slice(qc * 512, (qc + 1) * 512)
                        pvs = [ps_pv.tile([128, 512], f32, tag=f"pv{m}",
                                          name=f"pv{m}_{hl}_{qc}")
                               for m in range(5)]
                        zt = ps_pv.tile([1, 512], f32, tag="z", name=f"z{hl}_{qc}")
                        for kt in range(TOKT):
                            psS = ps_s.tile([128, 512], f32, tag="s",
                                            name=f"psS{hl}_{qc}_{kt}")
                            for dc in range(5):
                                nc.tensor.matmul(
                                    psS[:], kT[dc][:, kt * 128:(kt + 1) * 128],
                                    qT[dc][:, qsl],
                                    start=(dc == 0), stop=(dc == 4),
                                    skip_group_check=True)
                            es = expp.tile([128, 512], f32r, tag="expS",
                                           name=f"es{hl}_{qc}_{kt}")
                            nc.scalar.activation(es[:], psS[:], Exp)
                            for dc in range(5):
                                nc.tensor.matmul(
                                    pvs[dc][:], v[kt][:, dc * 128:(dc + 1) * 128],
                                    es[:], start=(kt == 0), stop=(kt == TOKT - 1),
                                    skip_group_check=True)
                            nc.tensor.matmul(
                                zt[:], ones_t[:], es[:],
                                start=(kt == 0), stop=(kt == TOKT - 1),
                                skip_group_check=True)
                        # normalize + evict to ag_in
                        rz = evp.tile([1, 512], f32, tag="rz", name=f"rz{hl}_{qc}")
                        nc.vector.reciprocal(rz[:], zt[:])
                        rzb = evp.tile([128, 512], f32, tag="rzb",
                                       name=f"rzb{hl}_{qc}")
                        nc.gpsimd.partition_broadcast(rzb[:], rz[:])
                        for dc in range(5):
                            ot = evp.tile([128, 512], bf16, tag="ot",
                                          name=f"ot{hl}_{qc}_{dc}")
                            nc.vector.tensor_mul(ot[:], pvs[dc][:], rzb[:])
                            nc.sync.dma_start(
                                ag_in[hl * 640 + dc * 128:hl * 640 + (dc + 1) * 128,
                                      qsl], ot[:])

            # ---------- AllGather over TP group ----------
            nc.gpsimd.collective_compute(
                "AllGather", mybir.AluOpType.bypass,
                replica_groups=[[0, 1, 2, 3], [4, 5, 6, 7]],
                ins=[ag_in.opt()], outs=[ag_out.opt()])

            # ---------- phase E: output projection ----------
            _inner.close()  # free const + cT SBUF before the slab
            with tc.tile_pool(name="slab", bufs=1) as slp, \
                 tc.tile_pool(name="wo", bufs=4) as wop, \
                 tc.tile_pool(name="oev", bufs=3) as oev, \
                 tc.tile_pool(name="bo", bufs=1) as bop, \
                 tc.tile_pool(name="psE", bufs=1, space="PSUM") as psE:
                bo_t = bop.tile([128, ESL], f32, name="bo_t")
                nc.sync.dma_start(bo_t[:], bo)
                slab = []
                for i in range(DM_CH):
                    s = slp.tile([128, SEQ], bf16, name=f"sl{i}")
                    nc.sync.dma_start(s[:], ag_out[i * 128:(i + 1) * 128, :])
                    slab.append(s)
                E_CH = [(0, 512), (512, 512), (1024, 256)]
                for eoff, ew in E_CH:
                    for th in range(2):
                        pse = [psE.tile([128, 512], f32, tag=f"e{t}",
                                        name=f"psE_{eoff}_{th}_{t}")
                               for t in range(8)]
                        for dm in range(DM_CH):
                            wo_t = wop.tile([128, 512], bf16, tag="wo",
                                            name=f"wo{eoff}_{th}_{dm}")
                            nc.sync.dma_start(wo_t[:, :ew],
                                              Wo[dm * 128:(dm + 1) * 128,
                                                 eoff:eoff + ew])
                            for t8 in range(8):
                                tok = th * 8 + t8
                                nc.tensor.matmul(
                                    pse[t8][:, :ew],
                                    slab[dm][:, tok * 128:(tok + 1) * 128],
                                    wo_t[:, :ew],
                                    start=(dm == 0), stop=(dm == DM_CH - 1),
                                    skip_group_check=True)
                        for t8 in range(8):
                            tok = th * 8 + t8
                            oe = oev.tile([128, 512], f32, tag="oe",
                                          name=f"oe{eoff}_{th}_{t8}")
                            nc.vector.tensor_add(oe[:, :ew], pse[t8][:, :ew],
                                                 bo_t[:, eoff:eoff + ew])
                            nc.sync.dma_start(
                                out[tok * 128:(tok + 1) * 128, eoff:eoff + ew],
                                oe[:, :ew])

    nc.compile()
    return nc


def _rope_tables():
    inv_freq = (1.0 / (10000.0 ** (np.arange(0, D_ROPE // 2, 2, dtype=np.float32)
                                   / (D_ROPE // 2)))).astype(np.float32)
    t = np.arange(SEQ, dtype=np.float32) / np.float32(ROPE_SCALE)
    freqs = t[:, None] * inv_freq[None, :]          # [SEQ, 4]
    cos = np.cos(freqs).astype(np.float32)          # [SEQ, 4]
    sin = np.sin(freqs).astype(np.float32)
    costab = np.ones((16, SEQ), np.float32)
    sintab = np.zeros((16, SEQ), np.float32)
    for p in range(16):
        f = p % 16
        j = p % 4
        if f < 8:
            costab[p] = cos[:, j]
        if f < 4:
            sintab[p] = -sin[:, j]
        elif f < 8:
            sintab[p] = sin[:, j]
    return costab, sintab


def _shard(inp):
    f32 = np.float32
    bf16 = ml_dtypes.bfloat16
    h = np.asarray(inp['h'], f32)
    W_dkv = np.asarray(inp['W_dkv'], f32); b_dkv = np.asarray(inp['b_dkv'], f32)
    W_dq = np.asarray(inp['W_dq'], f32); b_dq = np.asarray(inp['b_dq'], f32)
    W_uk = np.asarray(inp['W_uk'], f32); b_uk = np.asarray(inp['b_uk'], f32)
    W_uv = np.asarray(inp['W_uv'], f32); b_uv = np.asarray(inp['b_uv'], f32)
    W_uq = np.asarray(inp['W_uq'], f32); b_uq = np.asarray(inp['b_uq'], f32)
    W_qr = np.asarray(inp['W_qr'], f32); b_qr = np.asarray(inp['b_qr'], f32)
    W_kr = np.asarray(inp['W_kr'], f32); b_kr = np.asarray(inp['b_kr'], f32)
    W_o = np.asarray(inp['W_o'], f32); b_o = np.asarray(inp['b_o'], f32)

    costab, sintab = _rope_tables()
    down_bias_base = np.stack([b_dkv, b_dq], axis=1)  # [128,2]
    hTs = [np.ascontiguousarray(h[b].T) for b in range(BATCH)]

    in_maps = []
    for c in range(N_CORES):
        b, g = divmod(c, TP)
        heads = [2 * g, 2 * g + 1]
        # W_kr columns rotated so this core's two heads land at rows 0:32
        kr_cols = np.concatenate(
            [np.arange(hh * 16, hh * 16 + 16) for hh in heads] +
            [np.arange(hh * 16, hh * 16 + 16) for hh in range(N_HEADS)
             if hh not in heads])
        Wdown = np.concatenate([W_dkv, W_dq, W_kr[:, kr_cols]], axis=1)
        db = np.concatenate([down_bias_base, b_kr[kr_cols][:, None]], axis=1)
        Wq_c = np.concatenate(
            [np.concatenate([W_uq[:, hh * 624:(hh + 1) * 624],
                             W_qr[:, hh * 16:(hh + 1) * 16]], axis=1)
             for hh in heads], axis=1)
        Wk_c = np.concatenate([W_uk[:, hh * 624:(hh + 1) * 624] for hh in heads],
                              axis=1)
        Wv_c = np.concatenate([W_uv[:, hh * 640:(hh + 1) * 640] for hh in heads],
                              axis=1)
        qkb = np.zeros((128, 20), f32)
        for hl, hh in enumerate(heads):
            qb = np.concatenate([b_uq[hh * 624:(hh + 1) * 624],
                                 b_qr[hh * 16:(hh + 1) * 16]]) * SCALE
            qkb[:, hl * 5:(hl + 1) * 5] = qb.reshape(5, 128).T
            kb = np.zeros(640, f32)
            kb[:624] = b_uk[hh * 624:(hh + 1) * 624]
            qkb[:, 10 + hl * 5:10 + (hl + 1) * 5] = kb.reshape(5, 128).T
        vb = np.tile(np.concatenate(
            [b_uv[hh * 640:(hh + 1) * 640] for hh in heads])[None, :], (128, 1))
        esl = slice(g * ESL, (g + 1) * ESL)
        in_maps.append({
            "hT": hTs[b],
            "Wdown": np.ascontiguousarray(Wdown, f32),
            "down_bias": np.ascontiguousarray(db, f32),
            "Wq": np.ascontiguousarray(Wq_c, f32),
            "Wk": np.ascontiguousarray(Wk_c, f32),
            "Wv": np.ascontiguousarray(Wv_c, f32),
            "qk_bias": qkb,
            "vbias": np.ascontiguousarray(vb, f32),
            "ones": np.ones((128, 1), np.float32),
            "costab": costab,
            "sintab": sintab,
            "Wo": np.ascontiguousarray(W_o[:, esl]).astype(bf16),
            "bo": np.ascontiguousarray(
                np.tile(b_o[esl][None, :], (128, 1)), f32),
        })
    return in_maps


def kernel(**inputs):
    global LAST_RESULTS
    from concourse import bass_utils
    if 'nc' not in _CACHE:
        _CACHE['nc'] = _build_nc()
    nc = _CACHE['nc']
    in_maps = _shard(inputs)
    res = bass_utils.run_bass_kernel_spmd(nc, in_maps,
                                          core_ids=list(range(N_CORES)))
    LAST_RESULTS = res
    out = np.empty((BATCH, SEQ, D_MODEL), np.float32)
    for c in range(N_CORES):
        b, g = divmod(c, TP)
        out[b, :, g * ESL:(g + 1) * ESL] = res.results[c]["out"]
    return out


# revision 13
# speedup vs baseline: 1.7181x; 1.7181x over previous
"""Memory-optimized MLA (multi-head latent attention) Bass kernel for 8 trn2 cores.

Sharding: DP=2 over batch x TP=4 over heads (2 heads/core).
core c -> batch b=c//4, head-group g=c%4 (global heads 2g, 2g+1).

Per-core device pipeline (all layouts transposed: [feature, token]):
  A: down-proj  c_kvT/c_qT/k_rotT = W_down^T @ h^T      (K=d_model, 40 chunks)
  B: up-proj    qT/kT [640,2048], v [2048,640] per head  (K=d_kv=128)  + RoPE
  C: attention  S^T = kT^T qT -> exp -> PV + ones-row normalizer (no-max softmax)
  AG: AllGather normalized attn_out^T slices over the 4 TP ranks (bf16)
  E: out[tok, e_slice] = attn_full^T^T @ W_o[:, e_slice] + b_o   (bf16 matmul)
Host gathers the 8 [2048,1280] slices into [2,2048,5120].
"""
import sys
if '/opt/trn_rl_repo' not in sys.path:
    sys.path.insert(0, '/opt/trn_rl_repo')
import numpy as np
import ml_dtypes

D_MODEL = 5120
N_HEADS = 8
D_HEAD = 640
D_ROPE = 16
SPLIT = 624
D_KV = 128
BATCH, SEQ = 2, 2048
ROPE_SCALE = 40.0
SCALE = 1.0 / np.sqrt(np.float32(D_HEAD))

N_CORES = 8
TP = 4           # head-parallel ranks per batch group
HL = 2           # local heads per core
ESL = D_MODEL // TP          # 1280 output cols per core
DM_CH = D_MODEL // 128       # 40
TOKT = SEQ // 128            # 16 token tiles
TOKC = SEQ // 512            # 4 token chunks

_CACHE = {}
LAST_RESULTS = None


def _build_nc(with_ag=True):
    import concourse.bacc as bacc
    import concourse.mybir as mybir
    import concourse.tile as tile

    f32 = mybir.dt.float32
    f32r = mybir.dt.float32r
    bf16 = mybir.dt.bfloat16
    Exp = mybir.ActivationFunctionType.Exp
    Copy = mybir.ActivationFunctionType.Copy
    Ident = mybir.ActivationFunctionType.Identity

    nc = bacc.Bacc("TRN2", target_bir_lowering=False, debug=False,
                   num_devices=N_CORES)

    hT = nc.dram_tensor("hT", [D_MODEL, SEQ], f32r, kind="ExternalInput").ap()
    Wdown = nc.dram_tensor("Wdown", [D_MODEL, 384], f32r, kind="ExternalInput").ap()
    down_bias = nc.dram_tensor("down_bias", [128, 3], f32, kind="ExternalInput").ap()
    Wq = nc.dram_tensor("Wq", [D_KV, HL * 640], f32r, kind="ExternalInput").ap()
    Wk = nc.dram_tensor("Wk", [D_KV, HL * 624], f32r, kind="ExternalInput").ap()
    Wv = nc.dram_tensor("Wv", [D_KV, HL * 640], f32r, kind="ExternalInput").ap()
    qk_bias = nc.dram_tensor("qk_bias", [128, 20], f32, kind="ExternalInput").ap()
    vbias = nc.dram_tensor("vbias", [128, HL * 640], f32, kind="ExternalInput").ap()
    costab = nc.dram_tensor("costab", [16, SEQ], f32r, kind="ExternalInput").ap()
    sintab = nc.dram_tensor("sintab", [16, SEQ], f32r, kind="ExternalInput").ap()
    ones_in = nc.dram_tensor("ones", [128, 1], f32r, kind="ExternalInput").ap()
    Wo = nc.dram_tensor("Wo", [D_MODEL, ESL], bf16, kind="ExternalInput").ap()
    bo = nc.dram_tensor("bo", [128, ESL], f32, kind="ExternalInput").ap()
    out = nc.dram_tensor("out", [SEQ, ESL], f32, kind="ExternalOutput").ap()

    from contextlib import ExitStack
    with tile.TileContext(nc) as tc:
        # ---------- constants ----------
        with ExitStack() as _stk:
            dram = _stk.enter_context(
                tc.tile_pool(name="dram", bufs=1, space="DRAM"))
            _inner = ExitStack()
            cst = _inner.enter_context(tc.tile_pool(name="const", bufs=1))
            cTp = _inner.enter_context(tc.tile_pool(name="cT", bufs=1))
            dbias_t = cst.tile([128, 3], f32, name="dbias_t")
            nc.sync.dma_start(dbias_t[:], down_bias)
            qkb_t = cst.tile([128, 20], f32, name="qkb_t")
            nc.sync.dma_start(qkb_t[:], qk_bias)
            vb_t = cst.tile([128, HL * 640], f32, name="vb_t")
            nc.sync.dma_start(vb_t[:], vbias)
            ones_t = cst.tile([128, 1], f32r, name="ones_t")
            nc.sync.dma_start(ones_t[:], ones_in)
            # rope tables parked at partitions 112:128 (both rope sites land there)
            cs_t = cst.tile([128, SEQ], f32r, name="cs_t")
            sn_t = cst.tile([128, SEQ], f32r, name="sn_t")
            nc.sync.dma_start(cs_t[0:16, :], costab)
            nc.sync.dma_start(sn_t[0:16, :], sintab)

            c_kvT = cTp.tile([128, SEQ], f32r, name="c_kvT")
            c_qT = cTp.tile([128, SEQ], f32r, name="c_qT")
            k_rotT = cTp.tile([128, SEQ], f32r, name="k_rotT")
            cT = [c_kvT, c_qT, k_rotT]

            ag_in = [dram.tile([640, SEQ], bf16, name=f"ag_in{l}")
                     for l in range(HL)]
            ag_out = [dram.tile([TP * 640, SEQ], bf16, name=f"ag_out{l}")
                      for l in range(HL)]

            # ---------- phase A: down projections ----------
            with tc.tile_pool(name="wdown", bufs=1) as wdp, \
                 tc.tile_pool(name="hstream", bufs=4) as hsp, \
                 tc.tile_pool(name="psA", bufs=6, space="PSUM") as psA:
                wd = [None] * DM_CH
                for tc4 in range(TOKC):
                    ps = [psA.tile([128, 512], f32, tag="psA", name=f"psA_{tc4}_{m}")
                          for m in range(3)]
                    for dm in range(DM_CH):
                        if wd[dm] is None:
                            w = wdp.tile([128, 384], f32r, name=f"wd{dm}")
                            nc.sync.dma_start(w[:], Wdown[dm * 128:(dm + 1) * 128, :])
                            wd[dm] = w
                        ht = hsp.tile([128, 512], f32r, tag="ht", name=f"ht{tc4}_{dm}")
                        nc.scalar.dma_start(
                            ht[:], hT[dm * 128:(dm + 1) * 128,
                                      tc4 * 512:(tc4 + 1) * 512])
                        for m in range(3):
                            nc.tensor.matmul(
                                ps[m][:], wd[dm][:, m * 128:(m + 1) * 128],
                                ht[:], start=(dm == 0), stop=(dm == DM_CH - 1),
                                skip_group_check=True)
                    for m in range(3):
                        nc.scalar.activation(
                            cT[m][:, tc4 * 512:(tc4 + 1) * 512], ps[m][:],
                            Ident, bias=dbias_t[:, m:m + 1])

            # ---------- phases B+C per head ----------
            with tc.tile_pool(name="upw", bufs=1) as upw, \
                 tc.tile_pool(name="qkv", bufs=1) as qkv, \
                 tc.tile_pool(name="expp", bufs=6) as expp, \
                 tc.tile_pool(name="ev", bufs=2) as evp, \
                 tc.tile_pool(name="ps_s", bufs=2, space="PSUM") as ps_s, \
                 tc.tile_pool(name="ps_pv", bufs=1, space="PSUM") as ps_pv:
                for hl in range(HL):
                    wq_t = upw.tile([128, 640], f32r, tag="wq", name=f"wq{hl}")
                    nc.sync.dma_start(wq_t[:], Wq[:, hl * 640:(hl + 1) * 640])
                    wk_t = upw.tile([128, 624], f32r, tag="wk", name=f"wk{hl}")
                    nc.sync.dma_start(wk_t[:], Wk[:, hl * 624:(hl + 1) * 624])
                    wv_t = upw.tile([128, 640], f32r, tag="wv", name=f"wv{hl}")
                    nc.sync.dma_start(wv_t[:], Wv[:, hl * 640:(hl + 1) * 640])

                    qT = [qkv.tile([128, SEQ], f32r, tag=f"qT{m}", name=f"qT{m}_{hl}")
                          for m in range(5)]
                    kT = [qkv.tile([128, SEQ], f32r, tag=f"kT{m}", name=f"kT{m}_{hl}")
                          for m in range(5)]
                    v = [qkv.tile([128, D_HEAD], f32r, tag=f"v{t}", name=f"v{t}_{hl}")
                         for t in range(TOKT)]

                    # q up-proj (5 chunks of 128 feats; last 16 rows are q_rot)
                    for m in range(5):
                        for tc4 in range(TOKC):
                            ps = ps_s.tile([128, 512], f32, tag="s",
                                           name=f"psq{hl}_{m}_{tc4}")
                            nc.tensor.matmul(
                                ps[:], wq_t[:, m * 128:(m + 1) * 128],
                                c_qT[:, tc4 * 512:(tc4 + 1) * 512],
                                start=True, stop=True, skip_group_check=True)
                            nc.scalar.activation(
                                qT[m][:, tc4 * 512:(tc4 + 1) * 512], ps[:],
                                Ident, bias=qkb_t[:, hl * 5 + m:hl * 5 + m + 1],
                                scale=float(SCALE))
                    # k up-proj (624 feats: 4x128 + 112)
                    for m in range(5):
                        mw = 128 if m < 4 else 112
                        for tc4 in range(TOKC):
                            ps = ps_s.tile([128, 512], f32, tag="s",
                                           name=f"psk{hl}_{m}_{tc4}")
                            nc.tensor.matmul(
                                ps[:mw, :], wk_t[:, m * 128:m * 128 + mw],
                                c_kvT[:, tc4 * 512:(tc4 + 1) * 512],
                                start=True, stop=True, skip_group_check=True)
                            nc.scalar.activation(
                                kT[m][:mw, tc4 * 512:(tc4 + 1) * 512], ps[:mw, :],
                                Ident, bias=qkb_t[:mw, 10 + hl * 5 + m:11 + hl * 5 + m])
                    # k_rot rows -> kT[4][112:128]
                    h_glob = None  # head index resolved on host; rows via input layout
                    # k_rotT rows for this local head sit at [ (2g+hl)*16 : +16 ] --
                    # host packs k_rotT for ALL heads; we need global head = 2g+hl,
                    # but g differs per core. Host instead rotates W_kr columns so
                    # that OUR two heads occupy rows 0:16 (hl=0) and 16:32 (hl=1).
                    nc.sync.dma_start(kT[4][112:128, :],
                                      k_rotT[hl * 16:(hl + 1) * 16, :])
                    # v up-proj: [tok, d] orientation
                    for t in range(TOKT):
                        for half in range(2):
                            ps = ps_s.tile([128, 320], f32, tag="s",
                                           name=f"psv{hl}_{t}_{half}")
                            nc.tensor.matmul(
                                ps[:], c_kvT[:, t * 128:(t + 1) * 128],
                                wv_t[:, half * 320:(half + 1) * 320],
                                start=True, stop=True, skip_group_check=True)
                            nc.vector.tensor_add(
                                v[t][:, half * 320:(half + 1) * 320], ps[:],
                                vb_t[:, hl * 640 + half * 320:hl * 640 + (half + 1) * 320])

                    # RoPE on qT[4][112:128] / kT[4][112:128]: DVE needs
                    # 32-aligned partition bases, so stage rows at 0:16.
                    for name, x in (("q", qT[4]), ("k", kT[4])):
                        for cc in range(TOKC):
                            sl = slice(cc * 512, (cc + 1) * 512)
                            stg = expp.tile([128, 512], f32r, tag="expS",
                                            name=f"ropes_{name}{hl}_{cc}")
                            tmp = expp.tile([128, 512], f32r, tag="expS",
                                            name=f"ropet_{name}{hl}_{cc}")
                            nc.sync.dma_start(stg[0:16, 0:512], x[112:128, sl])
                            nc.sync.dma_start(tmp[0:4, 0:512], stg[4:8, 0:512])
                            nc.sync.dma_start(tmp[4:8, 0:512], stg[0:4, 0:512])
                            nc.vector.tensor_mul(tmp[0:8, 0:512], tmp[0:8, 0:512],
                                                 sn_t[0:8, sl])
                            nc.vector.tensor_mul(stg[0:8, 0:512], stg[0:8, 0:512],
                                                 cs_t[0:8, sl])
                            nc.vector.tensor_add(stg[0:8, 0:512], stg[0:8, 0:512],
                                                 tmp[0:8, 0:512])
                            nc.sync.dma_start(x[112:128, sl], stg[0:16, 0:512])

                    # ---------- phase C: attention ----------
                    for qc in range(TOKC):
                        qsl = slice(qc * 512, (qc + 1) * 512)
                        pvs = [ps_pv.tile([128, 512], f32, tag=f"pv{m}",
                                          name=f"pv{m}_{hl}_{qc}")
                               for m in range(5)]
                        zt = ps_pv.tile([1, 512], f32, tag="z", name=f"z{hl}_{qc}")
                        for kt in range(TOKT):
                            psS = ps_s.tile([128, 512], f32, tag="s",
                                            name=f"psS{hl}_{qc}_{kt}")
                            for dc in range(5):
                                nc.tensor.matmul(
                                    psS[:], kT[dc][:, kt * 128:(kt + 1) * 128],
                                    qT[dc][:, qsl],
                                    start=(dc == 0), stop=(dc == 4),
                                    skip_group_check=True)
                            es = expp.tile([128, 512], f32r, tag="expS",
                                           name=f"es{hl}_{qc}_{kt}")
                            nc.scalar.activation(es[:], psS[:], Exp)
                            for dc in range(5):
                                nc.tensor.matmul(
                                    pvs[dc][:], v[kt][:, dc * 128:(dc + 1) * 128],
                                    es[:], start=(kt == 0), stop=(kt == TOKT - 1),
                                    skip_group_check=True)
                            nc.tensor.matmul(
                                zt[:], ones_t[:], es[:],
                                start=(kt == 0), stop=(kt == TOKT - 1),
                                skip_group_check=True)
                        # normalize + evict to ag_in
                        rz = evp.tile([1, 512], f32, tag="rz", name=f"rz{hl}_{qc}")
                        nc.vector.reciprocal(rz[:], zt[:])
                        rzb = evp.tile([128, 512], f32, tag="rzb",
                                       name=f"rzb{hl}_{qc}")
                        nc.gpsimd.partition_broadcast(rzb[:], rz[:])
                        for dc in range(5):
                            ot = evp.tile([128, 512], bf16, tag="ot",
                                          name=f"ot{hl}_{qc}_{dc}")
                            nc.vector.tensor_mul(ot[:], pvs[dc][:], rzb[:])
                            nc.sync.dma_start(
                                ag_in[hl][dc * 128:(dc + 1) * 128, qsl], ot[:])

            # ---------- AllGather over TP group ----------
            for l in range(HL):
                if with_ag:
                    nc.gpsimd.collective_compute(
                        "AllGather", mybir.AluOpType.bypass,
                        replica_groups=[[0, 1, 2, 3], [4, 5, 6, 7]],
                        ins=[ag_in[l].opt()], outs=[ag_out[l].opt()])
                else:  # timing-only variant for TimelineSim (no collectives)
                    nc.sync.dma_start(ag_out[l][0:640, :], ag_in[l][:])

            # ---------- phase E: output projection ----------
            _inner.close()  # free const + cT SBUF before the slab
            with tc.tile_pool(name="slab", bufs=1) as slp, \
                 tc.tile_pool(name="wo", bufs=8) as wop, \
                 tc.tile_pool(name="oev", bufs=3) as oev, \
                 tc.tile_pool(name="bo", bufs=1) as bop, \
                 tc.tile_pool(name="psE", bufs=1, space="PSUM") as psE:
                bo_t = bop.tile([128, ESL], f32, name="bo_t")
                nc.sync.dma_start(bo_t[:], bo)
                # chunk i -> global head h=i//5, local head l=h%2, rank r=h//2
                dm_order = [i for i in range(DM_CH) if (i // 5) % 2 == 0] + \
                           [i for i in range(DM_CH) if (i // 5) % 2 == 1]
                slab = [None] * DM_CH
                for i in dm_order:
                    h = i // 5
                    l, r2, j = h % 2, h // 2, i % 5
                    st = slp.tile([128, SEQ], bf16, name=f"sl{i}")
                    nc.sync.dma_start(
                        st[:], ag_out[l][r2 * 640 + j * 128:r2 * 640 + (j + 1) * 128, :])
                    slab[i] = st
                E_CH = [(0, 512), (512, 512), (1024, 256)]
                for eoff, ew in E_CH:
                    for th in range(2):
                        pse = [psE.tile([128, 512], f32, tag=f"e{t}",
                                        name=f"psE_{eoff}_{th}_{t}")
                               for t in range(8)]
                        for di, dm in enumerate(dm_order):
                            wo_t = wop.tile([128, 512], bf16, tag="wo",
                                            name=f"wo{eoff}_{th}_{dm}")
                            nc.sync.dma_start(wo_t[:, :ew],
                                              Wo[dm * 128:(dm + 1) * 128,
                                                 eoff:eoff + ew])
                            for t8 in range(8):
                                tok = th * 8 + t8
                                nc.tensor.matmul(
                                    pse[t8][:, :ew],
                                    slab[dm][:, tok * 128:(tok + 1) * 128],
                                    wo_t[:, :ew],
                                    start=(di == 0), stop=(di == DM_CH - 1),
                                    skip_group_check=True)
                        for t8 in range(8):
                            tok = th * 8 + t8
                            oe = oev.tile([128, 512], f32, tag="oe",
                                          name=f"oe{eoff}_{th}_{t8}")
                            nc.vector.tensor_add(oe[:, :ew], pse[t8][:, :ew],
                                                 bo_t[:, eoff:eoff + ew])
                            nc.sync.dma_start(
                                out[tok * 128:(tok + 1) * 128, eoff:eoff + ew],
                                oe[:, :ew])

    nc.compile()
    return nc


def _rope_tables():
    inv_freq = (1.0 / (10000.0 ** (np.arange(0, D_ROPE // 2, 2, dtype=np.float32)
                                   / (D_ROPE // 2)))).astype(np.float32)
    t = np.arange(SEQ, dtype=np.float32) / np.float32(ROPE_SCALE)
    freqs = t[:, None] * inv_freq[None, :]          # [SEQ, 4]
    cos = np.cos(freqs).astype(np.float32)          # [SEQ, 4]
    sin = np.sin(freqs).astype(np.float32)
    costab = np.ones((16, SEQ), np.float32)
    sintab = np.zeros((16, SEQ), np.float32)
    for p in range(16):
        f = p % 16
        j = p % 4
        if f < 8:
            costab[p] = cos[:, j]
        if f < 4:
            sintab[p] = -sin[:, j]
        elif f < 8:
            sintab[p] = sin[:, j]
    return costab, sintab


def _shard(inp):
    f32 = np.float32
    bf16 = ml_dtypes.bfloat16
    h = np.asarray(inp['h'], f32)
    W_dkv = np.asarray(inp['W_dkv'], f32); b_dkv = np.asarray(inp['b_dkv'], f32)
    W_dq = np.asarray(inp['W_dq'], f32); b_dq = np.asarray(inp['b_dq'], f32)
    W_uk = np.asarray(inp['W_uk'], f32); b_uk = np.asarray(inp['b_uk'], f32)
    W_uv = np.asarray(inp['W_uv'], f32); b_uv = np.asarray(inp['b_uv'], f32)
    W_uq = np.asarray(inp['W_uq'], f32); b_uq = np.asarray(inp['b_uq'], f32)
    W_qr = np.asarray(inp['W_qr'], f32); b_qr = np.asarray(inp['b_qr'], f32)
    W_kr = np.asarray(inp['W_kr'], f32); b_kr = np.asarray(inp['b_kr'], f32)
    W_o = np.asarray(inp['W_o'], f32); b_o = np.asarray(inp['b_o'], f32)

    costab, sintab = _rope_tables()
    down_bias_base = np.stack([b_dkv, b_dq], axis=1)  # [128,2]
    hTs = [np.ascontiguousarray(h[b].T) for b in range(BATCH)]

    in_maps = []
    for c in range(N_CORES):
        b, g = divmod(c, TP)
        heads = [2 * g, 2 * g + 1]
        # W_kr columns rotated so this core's two heads land at rows 0:32
        kr_cols = np.concatenate(
            [np.arange(hh * 16, hh * 16 + 16) for hh in heads] +
            [np.arange(hh * 16, hh * 16 + 16) for hh in range(N_HEADS)
             if hh not in heads])
        Wdown = np.concatenate([W_dkv, W_dq, W_kr[:, kr_cols]], axis=1)
        db = np.concatenate([down_bias_base, b_kr[kr_cols][:, None]], axis=1)
        Wq_c = np.concatenate(
            [np.concatenate([W_uq[:, hh * 624:(hh + 1) * 624],
                             W_qr[:, hh * 16:(hh + 1) * 16]], axis=1)
             for hh in heads], axis=1)
        Wk_c = np.concatenate([W_uk[:, hh * 624:(hh + 1) * 624] for hh in heads],
                              axis=1)
        Wv_c = np.concatenate([W_uv[:, hh * 640:(hh + 1) * 640] for hh in heads],
                              axis=1)
        qkb = np.zeros((128, 20), f32)
        for hl, hh in enumerate(heads):
            qb = np.concatenate([b_uq[hh * 624:(hh + 1) * 624],
                                 b_qr[hh * 16:(hh + 1) * 16]]) * SCALE
            qkb[:, hl * 5:(hl + 1) * 5] = qb.reshape(5, 128).T
            kb = np.zeros(640, f32)
            kb[:624] = b_uk[hh * 624:(hh + 1) * 624]
            qkb[:, 10 + hl * 5:10 + (hl + 1) * 5] = kb.reshape(5, 128).T
        vb = np.tile(np.concatenate(
            [b_uv[hh * 640:(hh + 1) * 640] for hh in heads])[None, :], (128, 1))
        esl = slice(g * ESL, (g + 1) * ESL)
        in_maps.append({
            "hT": hTs[b],
            "Wdown": np.ascontiguousarray(Wdown, f32),
            "down_bias": np.ascontiguousarray(db, f32),
            "Wq": np.ascontiguousarray(Wq_c, f32),
            "Wk": np.ascontiguousarray(Wk_c, f32),
            "Wv": np.ascontiguousarray(Wv_c, f32),
            "qk_bias": qkb,
            "vbias": np.ascontiguousarray(vb, f32),
            "ones": np.ones((128, 1), np.float32),
            "costab": costab,
            "sintab": sintab,
            "Wo": np.ascontiguousarray(W_o[:, esl]).astype(bf16),
            "bo": np.ascontiguousarray(
                np.tile(b_o[esl][None, :], (128, 1)), f32),
        })
    return in_maps


def kernel(**inputs):
    global LAST_RESULTS
    from concourse import bass_utils
    if 'nc' not in _CACHE:
        _CACHE['nc'] = _build_nc()
    nc = _CACHE['nc']
    in_maps = _shard(inputs)
    res = bass_utils.run_bass_kernel_spmd(nc, in_maps,
                                          core_ids=list(range(N_CORES)))
    LAST_RESULTS = res
    out = np.empty((BATCH, SEQ, D_MODEL), np.float32)
    for c in range(N_CORES):
        b, g = divmod(c, TP)
        out[b, :, g * ESL:(g + 1) * ESL] = res.results[c]["out"]
    return out
